# revision 11
# baseline (speedup 1.0000x reference)
"""Trainium2 Bass kernel for nn_BaselineAttention_36172214567310.

Reference computation (note the einsum 'bhqk,bhkd->bhkd' bug: the "attention
output" is v scaled by the column-sums of the softmax matrix):

    qkv = x @ w_qkv                       # [b, s, 3*H*D]
    q, k, v = split(qkv)                  # per head
    P = softmax(q @ k^T / sqrt(D))        # [q, k] rows sum to 1
    colsum[k] = sum_q P[q, k]
    values[k, :] = v[k, :] * colsum_h[k]
    out = values @ w_o

Sharding: 8 cores = 2 batches x 4 head-groups (4 heads each). Each core
computes qkv^T for its heads, scores + exp (fused rowsum on ACT) + colsum
matvec per head, scales v^T, and a partial out = values @ w_o_slice.
Host sums the 4 partials per batch.

All matmuls run in float32r (fp32 rounded to 11-bit mantissa; full PE rate).
"""

import sys

sys.path.insert(0, "/opt/trn_rl_repo")

import numpy as np

B, S, HIDDEN = 2, 2048, 1024
NH, HD = 16, 64
HPC = 4            # heads per core
FPC = 3 * HPC * HD # qkv feature columns per core (768)
N_CORES = 8
P = 128
NT = S // 512      # 512-column tiles over the sequence (4)
QC = S // P        # 128-row q chunks (16)


def round_f32r(a: np.ndarray) -> np.ndarray:
    """Round fp32 to the fp32r grid (11-bit mantissa, round-to-nearest-even)."""
    u = np.ascontiguousarray(a).view(np.uint32)
    low = u & np.uint32(0x00000FFF)
    base = u & np.uint32(0xFFFFF000)
    lsb = (u >> np.uint32(12)) & np.uint32(1)
    round_up = (low > 0x800) | ((low == 0x800) & (lsb == 1))
    out = base + (round_up.astype(np.uint32) << np.uint32(12))
    return out.view(np.float32)


_CACHE = {}


def _build():
    if "nc" in _CACHE:
        return _CACHE["nc"]

    import concourse.bass as bass
    import concourse.mybir as mybir
    import concourse.tile as tile
    from concourse import bacc
    from concourse.tile_rust import add_dep_helper

    F32 = mybir.dt.float32
    F32R = mybir.dt.float32r
    BF16 = mybir.dt.bfloat16
    EXP = mybir.ActivationFunctionType.Exp

    nc = bacc.Bacc()
    xT_d = nc.declare_dram_parameter("xT", [HIDDEN, S], F32R, isOutput=False)
    xTb_d = nc.declare_dram_parameter("xTb", [HIDDEN, S], BF16, isOutput=False)
    wq_d = nc.declare_dram_parameter("wq", [HIDDEN, HPC * HD], F32R, isOutput=False)
    wqb_d = nc.declare_dram_parameter("wqb", [HIDDEN, 2 * HPC * HD], BF16,
                                      isOutput=False)
    wo_d = nc.declare_dram_parameter("wo", [HPC * HD, HIDDEN], F32R, isOutput=False)
    out_d = nc.declare_dram_parameter("out", [S, HIDDEN], F32, isOutput=True)

    with tile.TileContext(nc) as tc:
        # persistent pools
        with tc.tile_pool(name="qkvt", bufs=1) as qkvt_pool, \
             tc.tile_pool(name="wq", bufs=1) as wq_pool, \
             tc.tile_pool(name="wo", bufs=1) as wo_pool:

            # ---- phase 1: qkv^T = (x @ w_qkv)^T for this core's heads ----
            # qkvT tiles: mc 0,1 = Q (2 heads each), 2,3 = K, 4,5 = V
            qkvt = [qkvt_pool.tile([P, S], BF16 if mc < 4 else F32R,
                                   name=f"qkvt{mc}") for mc in range(6)]
            wq_t = [wq_pool.tile([P, HPC * HD], F32R, name=f"wq{kc}")
                    for kc in range(8)]
            wqb_t = [wq_pool.tile([P, 2 * HPC * HD], BF16, name=f"wqb{kc}")
                     for kc in range(8)]
            for kc in range(8):
                nc.sync.dma_start(out=wq_t[kc], in_=wq_d[kc * P:(kc + 1) * P, :])
                nc.sync.dma_start(out=wqb_t[kc], in_=wqb_d[kc * P:(kc + 1) * P, :])
            wo_t = [wo_pool.tile([P, HIDDEN], F32R, name=f"wo{kc}") for kc in range(2)]
            for kc in range(2):
                nc.sync.dma_start(out=wo_t[kc], in_=wo_d[kc * P:(kc + 1) * P, :])

            with tc.tile_pool(name="xt", bufs=1) as xt_pool, \
                 tc.tile_pool(name="ps_qkv", bufs=4, space="PSUM") as ps_qkv:
                xtb = [xt_pool.tile([P, S], BF16, name=f"xtb{kc}") for kc in range(8)]
                for kc in range(8):
                    nc.sync.dma_start(out=xtb[kc], in_=xTb_d[kc * P:(kc + 1) * P, :])
                xt = [xt_pool.tile([P, S], F32R, name=f"xt{kc}") for kc in range(8)]
                for kc in range(8):
                    nc.sync.dma_start(out=xt[kc], in_=xT_d[kc * P:(kc + 1) * P, :])
                # Q, K projections in bf16 (mc 0-3), V in f32r (mc 4, 5)
                for mc in range(6):
                    for nt in range(NT):
                        ps = ps_qkv.tile([P, 512], F32, name="psq")
                        for kc in range(8):
                            if mc < 4:
                                nc.tensor.matmul(
                                    ps, wqb_t[kc][:, mc * P:(mc + 1) * P],
                                    xtb[kc][:, nt * 512:(nt + 1) * 512],
                                    start=(kc == 0), stop=(kc == 7))
                            else:
                                nc.tensor.matmul(
                                    ps, wq_t[kc][:, (mc - 4) * P:(mc - 3) * P],
                                    xt[kc][:, nt * 512:(nt + 1) * 512],
                                    start=(kc == 0), stop=(kc == 7))
                        nc.vector.tensor_copy(
                            out=qkvt[mc][:, nt * 512:(nt + 1) * 512], in_=ps)

            # ---- phase 2+3: per-head colsums (replicated across 64
            # partitions via a replicated matvec lhsT), then v^T *= colsum ----
            H = S // 2  # 1024-col half chunks so scores double-buffer in PSUM
            with tc.tile_pool(name="esb", bufs=4) as e_pool, \
                 tc.tile_pool(name="rs", bufs=6) as rs_pool, \
                 tc.tile_pool(name="ps_s", bufs=2, space="PSUM") as ps_s_pool, \
                 tc.tile_pool(name="ps_c", bufs=1, space="PSUM") as ps_c_pool:
                for j in range(HPC):
                    # colsum for head j, replicated across all 128 partitions
                    # (f32r matmuls require dst base partition 0)
                    qt = qkvt[j // 2]
                    kt = qkvt[2 + j // 2]
                    vt = qkvt[4 + j // 2]
                    bp = (j % 2) * 64
                    ps_c = ps_c_pool.tile([P, S], F32, name="psc")
                    for qc in range(QC):
                        e_sb = []
                        rs_h = []
                        for hh in range(2):
                            ps_s = ps_s_pool.tile([P, H], F32, name="pss")
                            for nt in range(2):
                                nc.tensor.matmul(
                                    ps_s[:, nt * 512:(nt + 1) * 512],
                                    qt[bp:bp + 64, qc * P:(qc + 1) * P],
                                    kt[bp:bp + 64,
                                       hh * H + nt * 512:hh * H + (nt + 1) * 512],
                                    start=True, stop=True)
                            e = e_pool.tile([P, H], BF16, name="esb")
                            r = rs_pool.tile([P, 1], F32, name=f"rs{hh}")
                            # E = exp(scores / 8), rowsum fused on ACT
                            nc.scalar.activation(e, ps_s, EXP, scale=0.125,
                                                 accum_out=r)
                            e_sb.append(e)
                            rs_h.append(r)
                        rs = rs_pool.tile([P, 1], F32, name="rs")
                        nc.vector.tensor_tensor(rs, rs_h[0], rs_h[1],
                                                mybir.AluOpType.add)
                        wr = rs_pool.tile([P, P], BF16, name="wr")
                        nc.vector.reciprocal(rs, rs)
                        nc.vector.tensor_copy(
                            out=wr, in_=rs.to_broadcast([P, P]))
                        for hh in range(2):
                            for nt in range(2):
                                nc.tensor.matmul(
                                    ps_c[:, hh * H + nt * 512:hh * H + (nt + 1) * 512],
                                    wr,
                                    e_sb[hh][:, nt * 512:(nt + 1) * 512],
                                    start=(qc == 0), stop=(qc == QC - 1))
                    # v^T *= colsum for this head's 64 partitions
                    nc.vector.tensor_tensor(
                        vt[bp:bp + 64, :], vt[bp:bp + 64, :],
                        ps_c[bp:bp + 64, :], mybir.AluOpType.mult)

            # ---- phase 4: out_partial = values @ w_o_slice  [s, hidden] ----
            with tc.tile_pool(name="osb", bufs=3) as o_pool, \
                 tc.tile_pool(name="ps_o", bufs=2, space="PSUM") as ps_o_pool:
                for sc in range(QC):
                    ps_o = ps_o_pool.tile([P, HIDDEN], F32, name="pso")
                    for nh in range(2):
                        for kc in range(2):
                            nc.tensor.matmul(
                                ps_o[:, nh * 512:(nh + 1) * 512],
                                qkvt[4 + kc][:, sc * P:(sc + 1) * P],
                                wo_t[kc][:, nh * 512:(nh + 1) * 512],
                                start=(kc == 0), stop=(kc == 1))
                    o_sb = o_pool.tile([P, HIDDEN], F32, name="osb")
                    nc.vector.tensor_copy(out=o_sb, in_=ps_o)
                    nc.sync.dma_start(out=out_d[sc * P:(sc + 1) * P, :], in_=o_sb)

    nc.compile()
    _CACHE["nc"] = nc
    return nc


def kernel(x: np.ndarray, w_qkv: np.ndarray, w_o: np.ndarray) -> np.ndarray:
    import ml_dtypes
    from concourse.bass_utils import run_bass_kernel_spmd

    nc = _build()

    xT = [round_f32r(np.ascontiguousarray(x[b].T)) for b in range(B)]
    xTb = [np.ascontiguousarray(x[b].T).astype(ml_dtypes.bfloat16) for b in range(B)]
    in_maps = []
    for c in range(N_CORES):
        b, g = divmod(c, HPC)
        wqk_slice = np.concatenate(
            [w_qkv[:, t * NH * HD + 256 * g: t * NH * HD + 256 * g + 256]
             for t in range(2)], axis=1)
        wv_slice = w_qkv[:, 2 * NH * HD + 256 * g: 2 * NH * HD + 256 * g + 256]
        wo_slice = w_o[256 * g:256 * g + 256, :]
        in_maps.append({
            "xT": xT[b],
            "xTb": xTb[b],
            "wq": round_f32r(wv_slice),
            "wqb": wqk_slice.astype(ml_dtypes.bfloat16),
            "wo": round_f32r(wo_slice),
        })

    res = run_bass_kernel_spmd(nc, in_maps, list(range(N_CORES)),
                               **_CACHE.get("run_kwargs", {}))
    _CACHE["last_result"] = res

    out = np.zeros((B, S, HIDDEN), np.float32)
    for c in range(N_CORES):
        out[c // HPC] += res.results[c]["out"]
    return out


# revision 12
# speedup vs baseline: 1.0029x; 1.0029x over previous
"""Trainium2 Bass kernel for nn_BaselineAttention_36172214567310.

Reference computation (note the einsum 'bhqk,bhkd->bhkd' bug: the "attention
output" is v scaled by the column-sums of the softmax matrix):

    qkv = x @ w_qkv                       # [b, s, 3*H*D]
    q, k, v = split(qkv)                  # per head
    P = softmax(q @ k^T / sqrt(D))        # [q, k] rows sum to 1
    colsum[k] = sum_q P[q, k]
    values[k, :] = v[k, :] * colsum_h[k]
    out = values @ w_o

Sharding: 8 cores = 2 batches x 4 head-groups (4 heads each). Each core
computes qkv^T for its heads, scores + exp (fused rowsum on ACT) + colsum
matvec per head, scales v^T, and a partial out = values @ w_o_slice.
Host sums the 4 partials per batch.

All matmuls run in float32r (fp32 rounded to 11-bit mantissa; full PE rate).
"""

import sys

sys.path.insert(0, "/opt/trn_rl_repo")

import numpy as np

B, S, HIDDEN = 2, 2048, 1024
NH, HD = 16, 64
HPC = 4            # heads per core
FPC = 3 * HPC * HD # qkv feature columns per core (768)
N_CORES = 8
P = 128
NT = S // 512      # 512-column tiles over the sequence (4)
QC = S // P        # 128-row q chunks (16)


def round_f32r(a: np.ndarray) -> np.ndarray:
    """Round fp32 to the fp32r grid (11-bit mantissa, round-to-nearest-even)."""
    u = np.ascontiguousarray(a).view(np.uint32)
    low = u & np.uint32(0x00000FFF)
    base = u & np.uint32(0xFFFFF000)
    lsb = (u >> np.uint32(12)) & np.uint32(1)
    round_up = (low > 0x800) | ((low == 0x800) & (lsb == 1))
    out = base + (round_up.astype(np.uint32) << np.uint32(12))
    return out.view(np.float32)


_CACHE = {}


def _build():
    if "nc" in _CACHE:
        return _CACHE["nc"]

    import concourse.bass as bass
    import concourse.mybir as mybir
    import concourse.tile as tile
    from concourse import bacc
    from concourse.tile_rust import add_dep_helper

    F32 = mybir.dt.float32
    F32R = mybir.dt.float32r
    BF16 = mybir.dt.bfloat16
    EXP = mybir.ActivationFunctionType.Exp

    nc = bacc.Bacc()
    xT_d = nc.declare_dram_parameter("xT", [HIDDEN, S], F32R, isOutput=False)
    xTb_d = nc.declare_dram_parameter("xTb", [HIDDEN, S], BF16, isOutput=False)
    wq_d = nc.declare_dram_parameter("wq", [HIDDEN, HPC * HD], F32R, isOutput=False)
    wqb_d = nc.declare_dram_parameter("wqb", [HIDDEN, 2 * HPC * HD], BF16,
                                      isOutput=False)
    wo_d = nc.declare_dram_parameter("wo", [HPC * HD, HIDDEN], F32R, isOutput=False)
    out_d = nc.declare_dram_parameter("out", [S, HIDDEN], F32, isOutput=True)

    with tile.TileContext(nc) as tc:
        # persistent pools
        with tc.tile_pool(name="qkvt", bufs=1) as qkvt_pool, \
             tc.tile_pool(name="wq", bufs=1) as wq_pool, \
             tc.tile_pool(name="wo", bufs=1) as wo_pool:

            # ---- phase 1: qkv^T = (x @ w_qkv)^T for this core's heads ----
            # qkvT tiles: mc 0,1 = Q (2 heads each), 2,3 = K, 4,5 = V
            qkvt = [qkvt_pool.tile([P, S], BF16 if mc < 4 else F32R,
                                   name=f"qkvt{mc}") for mc in range(6)]
            wq_t = [wq_pool.tile([P, HPC * HD], F32R, name=f"wq{kc}")
                    for kc in range(8)]
            wqb_t = [wq_pool.tile([P, 2 * HPC * HD], BF16, name=f"wqb{kc}")
                     for kc in range(8)]
            for kc in range(8):
                nc.sync.dma_start(out=wq_t[kc], in_=wq_d[kc * P:(kc + 1) * P, :])
                nc.sync.dma_start(out=wqb_t[kc], in_=wqb_d[kc * P:(kc + 1) * P, :])
            wo_t = [wo_pool.tile([P, HIDDEN], F32R, name=f"wo{kc}") for kc in range(2)]
            for kc in range(2):
                nc.sync.dma_start(out=wo_t[kc], in_=wo_d[kc * P:(kc + 1) * P, :])

            with tc.tile_pool(name="xt", bufs=1) as xt_pool, \
                 tc.tile_pool(name="ps_qkv", bufs=4, space="PSUM") as ps_qkv:
                xtb = [xt_pool.tile([P, S], BF16, name=f"xtb{kc}") for kc in range(8)]
                for kc in range(8):
                    nc.sync.dma_start(out=xtb[kc], in_=xTb_d[kc * P:(kc + 1) * P, :])
                xt = [xt_pool.tile([P, S], F32R, name=f"xt{kc}") for kc in range(8)]
                for kc in range(8):
                    nc.sync.dma_start(out=xt[kc], in_=xT_d[kc * P:(kc + 1) * P, :])
                # Q, K projections in bf16 (mc 0-3), V in f32r (mc 4, 5)
                for mc in range(6):
                    for nt in range(NT):
                        ps = ps_qkv.tile([P, 512], F32, name="psq")
                        for kc in range(8):
                            if mc < 4:
                                nc.tensor.matmul(
                                    ps, wqb_t[kc][:, mc * P:(mc + 1) * P],
                                    xtb[kc][:, nt * 512:(nt + 1) * 512],
                                    start=(kc == 0), stop=(kc == 7))
                            else:
                                nc.tensor.matmul(
                                    ps, wq_t[kc][:, (mc - 4) * P:(mc - 3) * P],
                                    xt[kc][:, nt * 512:(nt + 1) * 512],
                                    start=(kc == 0), stop=(kc == 7))
                        nc.vector.tensor_copy(
                            out=qkvt[mc][:, nt * 512:(nt + 1) * 512], in_=ps)

            # ---- phase 2+3: per-head colsums (replicated across 64
            # partitions via a replicated matvec lhsT), then v^T *= colsum ----
            H = S // 2  # 1024-col half chunks so scores double-buffer in PSUM
            with tc.tile_pool(name="esb", bufs=6) as e_pool, \
                 tc.tile_pool(name="rs", bufs=8) as rs_pool, \
                 tc.tile_pool(name="ps_s", bufs=2, space="PSUM") as ps_s_pool, \
                 tc.tile_pool(name="ps_c", bufs=1, space="PSUM") as ps_c_pool:
                for j in range(HPC):
                    # colsum for head j, replicated across all 128 partitions
                    # (f32r matmuls require dst base partition 0)
                    qt = qkvt[j // 2]
                    kt = qkvt[2 + j // 2]
                    vt = qkvt[4 + j // 2]
                    bp = (j % 2) * 64
                    ps_c = ps_c_pool.tile([P, S], F32, name="psc")
                    for qc in range(QC):
                        e_sb = []
                        rs_h = []
                        for hh in range(2):
                            ps_s = ps_s_pool.tile([P, H], F32, name="pss")
                            for nt in range(2):
                                nc.tensor.matmul(
                                    ps_s[:, nt * 512:(nt + 1) * 512],
                                    qt[bp:bp + 64, qc * P:(qc + 1) * P],
                                    kt[bp:bp + 64,
                                       hh * H + nt * 512:hh * H + (nt + 1) * 512],
                                    start=True, stop=True)
                            e = e_pool.tile([P, H], BF16, name="esb")
                            r = rs_pool.tile([P, 1], F32, name=f"rs{hh}")
                            # E = exp(scores / 8), rowsum fused on ACT
                            nc.scalar.activation(e, ps_s, EXP, scale=0.125,
                                                 accum_out=r)
                            e_sb.append(e)
                            rs_h.append(r)
                        rs = rs_pool.tile([P, 1], F32, name="rs")
                        nc.vector.tensor_tensor(rs, rs_h[0], rs_h[1],
                                                mybir.AluOpType.add)
                        wr = rs_pool.tile([P, P], BF16, name="wr")
                        nc.vector.reciprocal(rs, rs)
                        nc.vector.tensor_copy(
                            out=wr, in_=rs.to_broadcast([P, P]))
                        for hh in range(2):
                            for nt in range(2):
                                nc.tensor.matmul(
                                    ps_c[:, hh * H + nt * 512:hh * H + (nt + 1) * 512],
                                    wr,
                                    e_sb[hh][:, nt * 512:(nt + 1) * 512],
                                    start=(qc == 0), stop=(qc == QC - 1))
                    # v^T *= colsum for this head's 64 partitions
                    nc.vector.tensor_tensor(
                        vt[bp:bp + 64, :], vt[bp:bp + 64, :],
                        ps_c[bp:bp + 64, :], mybir.AluOpType.mult)

            # ---- phase 4: out_partial = values @ w_o_slice  [s, hidden] ----
            with tc.tile_pool(name="osb", bufs=3) as o_pool, \
                 tc.tile_pool(name="ps_o", bufs=2, space="PSUM") as ps_o_pool:
                for sc in range(QC):
                    ps_o = ps_o_pool.tile([P, HIDDEN], F32, name="pso")
                    for nh in range(2):
                        for kc in range(2):
                            nc.tensor.matmul(
                                ps_o[:, nh * 512:(nh + 1) * 512],
                                qkvt[4 + kc][:, sc * P:(sc + 1) * P],
                                wo_t[kc][:, nh * 512:(nh + 1) * 512],
                                start=(kc == 0), stop=(kc == 1))
                    o_sb = o_pool.tile([P, HIDDEN], F32, name="osb")
                    nc.vector.tensor_copy(out=o_sb, in_=ps_o)
                    nc.sync.dma_start(out=out_d[sc * P:(sc + 1) * P, :], in_=o_sb)

    nc.compile()
    _CACHE["nc"] = nc
    return nc


def kernel(x: np.ndarray, w_qkv: np.ndarray, w_o: np.ndarray) -> np.ndarray:
    import ml_dtypes
    from concourse.bass_utils import run_bass_kernel_spmd

    nc = _build()

    xT = [round_f32r(np.ascontiguousarray(x[b].T)) for b in range(B)]
    xTb = [np.ascontiguousarray(x[b].T).astype(ml_dtypes.bfloat16) for b in range(B)]
    in_maps = []
    for c in range(N_CORES):
        b, g = divmod(c, HPC)
        wqk_slice = np.concatenate(
            [w_qkv[:, t * NH * HD + 256 * g: t * NH * HD + 256 * g + 256]
             for t in range(2)], axis=1)
        wv_slice = w_qkv[:, 2 * NH * HD + 256 * g: 2 * NH * HD + 256 * g + 256]
        wo_slice = w_o[256 * g:256 * g + 256, :]
        in_maps.append({
            "xT": xT[b],
            "xTb": xTb[b],
            "wq": round_f32r(wv_slice),
            "wqb": wqk_slice.astype(ml_dtypes.bfloat16),
            "wo": round_f32r(wo_slice),
        })

    res = run_bass_kernel_spmd(nc, in_maps, list(range(N_CORES)),
                               **_CACHE.get("run_kwargs", {}))
    _CACHE["last_result"] = res

    out = np.zeros((B, S, HIDDEN), np.float32)
    for c in range(N_CORES):
        out[c // HPC] += res.results[c]["out"]
    return out


# revision 13
# speedup vs baseline: 1.1282x; 1.1250x over previous
"""Trainium2 Bass kernel for nn_BaselineAttention_36172214567310.

Reference computation (note the einsum 'bhqk,bhkd->bhkd' bug: the "attention
output" is v scaled by the column-sums of the softmax matrix):

    qkv = x @ w_qkv                       # [b, s, 3*H*D]
    q, k, v = split(qkv)                  # per head
    P = softmax(q @ k^T / sqrt(D))        # [q, k] rows sum to 1
    colsum[k] = sum_q P[q, k]
    values[k, :] = v[k, :] * colsum_h[k]
    out = values @ w_o

Sharding: 8 cores = 2 batches x 4 head-groups (4 heads each). Each core
computes qkv^T for its heads, scores + exp (fused rowsum on ACT) + colsum
matvec per head, scales v^T, and a partial out = values @ w_o_slice.
Host sums the 4 partials per batch.

All matmuls run in float32r (fp32 rounded to 11-bit mantissa; full PE rate).
"""

import sys

sys.path.insert(0, "/opt/trn_rl_repo")

import numpy as np

B, S, HIDDEN = 2, 2048, 1024
NH, HD = 16, 64
HPC = 4            # heads per core
FPC = 3 * HPC * HD # qkv feature columns per core (768)
N_CORES = 8
P = 128
NT = S // 512      # 512-column tiles over the sequence (4)
QC = S // P        # 128-row q chunks (16)


def round_f32r(a: np.ndarray) -> np.ndarray:
    """Round fp32 to the fp32r grid (11-bit mantissa, round-to-nearest-even)."""
    u = np.ascontiguousarray(a).view(np.uint32)
    low = u & np.uint32(0x00000FFF)
    base = u & np.uint32(0xFFFFF000)
    lsb = (u >> np.uint32(12)) & np.uint32(1)
    round_up = (low > 0x800) | ((low == 0x800) & (lsb == 1))
    out = base + (round_up.astype(np.uint32) << np.uint32(12))
    return out.view(np.float32)


_CACHE = {}


def _build():
    if "nc" in _CACHE:
        return _CACHE["nc"]

    import concourse.bass as bass
    import concourse.mybir as mybir
    import concourse.tile as tile
    from concourse import bacc
    from concourse.tile_rust import add_dep_helper

    F32 = mybir.dt.float32
    F32R = mybir.dt.float32r
    BF16 = mybir.dt.bfloat16
    EXP = mybir.ActivationFunctionType.Exp

    nc = bacc.Bacc()
    xT_d = nc.declare_dram_parameter("xT", [HIDDEN, S], F32R, isOutput=False)
    xTb_d = nc.declare_dram_parameter("xTb", [HIDDEN, S], BF16, isOutput=False)
    wq_d = nc.declare_dram_parameter("wq", [HIDDEN, HPC * HD], F32R, isOutput=False)
    wqb_d = nc.declare_dram_parameter("wqb", [HIDDEN, 2 * HPC * HD], BF16,
                                      isOutput=False)
    wo_d = nc.declare_dram_parameter("wo", [HPC * HD, HIDDEN], F32R, isOutput=False)
    out_d = nc.declare_dram_parameter("out", [S, HIDDEN], F32, isOutput=True)

    with tile.TileContext(nc) as tc:
        # persistent pools
        with tc.tile_pool(name="qkvt", bufs=1) as qkvt_pool, \
             tc.tile_pool(name="wq", bufs=1) as wq_pool, \
             tc.tile_pool(name="wo", bufs=1) as wo_pool:

            # ---- phase 1: qkv^T = (x @ w_qkv)^T for this core's heads ----
            # qkvT tiles: mc 0,1 = Q (2 heads each), 2,3 = K, 4,5 = V
            qkvt = [qkvt_pool.tile([P, S], BF16 if mc < 4 else F32R,
                                   name=f"qkvt{mc}") for mc in range(6)]
            wq_t = [wq_pool.tile([P, HPC * HD], F32R, name=f"wq{kc}")
                    for kc in range(8)]
            wqb_t = [wq_pool.tile([P, 2 * HPC * HD], BF16, name=f"wqb{kc}")
                     for kc in range(8)]
            for kc in range(8):
                nc.sync.dma_start(out=wq_t[kc], in_=wq_d[kc * P:(kc + 1) * P, :])
                nc.sync.dma_start(out=wqb_t[kc], in_=wqb_d[kc * P:(kc + 1) * P, :])
            wo_t = [wo_pool.tile([P, HIDDEN], F32R, name=f"wo{kc}") for kc in range(2)]
            for kc in range(2):
                nc.sync.dma_start(out=wo_t[kc], in_=wo_d[kc * P:(kc + 1) * P, :])

            with tc.tile_pool(name="xt", bufs=1) as xt_pool, \
                 tc.tile_pool(name="ps_qkv", bufs=4, space="PSUM") as ps_qkv:
                xtb = [xt_pool.tile([P, S], BF16, name=f"xtb{kc}") for kc in range(8)]
                for kc in range(8):
                    nc.sync.dma_start(out=xtb[kc], in_=xTb_d[kc * P:(kc + 1) * P, :])
                xt = [xt_pool.tile([P, S], F32R, name=f"xt{kc}") for kc in range(8)]
                for kc in range(8):
                    nc.sync.dma_start(out=xt[kc], in_=xT_d[kc * P:(kc + 1) * P, :])
                # Q, K projections in bf16 (mc 0-3), V in f32r (mc 4, 5)
                for mc in range(6):
                    for nt in range(NT):
                        ps = ps_qkv.tile([P, 512], F32, name="psq")
                        for kc in range(8):
                            if mc < 4:
                                nc.tensor.matmul(
                                    ps, wqb_t[kc][:, mc * P:(mc + 1) * P],
                                    xtb[kc][:, nt * 512:(nt + 1) * 512],
                                    start=(kc == 0), stop=(kc == 7))
                            else:
                                nc.tensor.matmul(
                                    ps, wq_t[kc][:, (mc - 4) * P:(mc - 3) * P],
                                    xt[kc][:, nt * 512:(nt + 1) * 512],
                                    start=(kc == 0), stop=(kc == 7))
                        nc.vector.tensor_copy(
                            out=qkvt[mc][:, nt * 512:(nt + 1) * 512], in_=ps)

            # ---- phase 2+3: per-head colsums (replicated across 64
            # partitions via a replicated matvec lhsT), then v^T *= colsum ----
            H = S // 2  # 1024-col half chunks so scores double-buffer in PSUM
            with tc.tile_pool(name="esb", bufs=6) as e_pool, \
                 tc.tile_pool(name="rs", bufs=8) as rs_pool, \
                 tc.tile_pool(name="ps_s", bufs=2, space="PSUM") as ps_s_pool, \
                 tc.tile_pool(name="ps_c", bufs=1, space="PSUM") as ps_c_pool:
                for j in range(HPC):
                    # colsum for head j, replicated across all 128 partitions
                    # (f32r matmuls require dst base partition 0)
                    qt = qkvt[j // 2]
                    kt = qkvt[2 + j // 2]
                    vt = qkvt[4 + j // 2]
                    bp = (j % 2) * 64
                    ps_c = ps_c_pool.tile([P, S], F32, name="psc")

                    def emit_matvec(pend):
                        wr_p, e_p, qc_p = pend
                        for hh in range(2):
                            for nt in range(2):
                                nc.tensor.matmul(
                                    ps_c[:, hh * H + nt * 512:
                                         hh * H + (nt + 1) * 512],
                                    wr_p,
                                    e_p[hh][:, nt * 512:(nt + 1) * 512],
                                    start=(qc_p == 0), stop=(qc_p == QC - 1))

                    pending = None  # software-pipelined matvec of qc-1
                    for qc in range(QC):
                        e_sb = []
                        rs_h = []
                        for hh in range(2):
                            ps_s = ps_s_pool.tile([P, H], F32, name="pss")
                            for nt in range(2):
                                nc.tensor.matmul(
                                    ps_s[:, nt * 512:(nt + 1) * 512],
                                    qt[bp:bp + 64, qc * P:(qc + 1) * P],
                                    kt[bp:bp + 64,
                                       hh * H + nt * 512:hh * H + (nt + 1) * 512],
                                    start=True, stop=True)
                            e = e_pool.tile([P, H], BF16, name="esb")
                            r = rs_pool.tile([P, 1], F32, name=f"rs{hh}")
                            # E = exp(scores / 8), rowsum fused on ACT
                            nc.scalar.activation(e, ps_s, EXP, scale=0.125,
                                                 accum_out=r)
                            e_sb.append(e)
                            rs_h.append(r)
                        # emit the PREVIOUS chunk's matvec here so PE can run
                        # it while ACT works on this chunk's exps
                        if pending is not None:
                            emit_matvec(pending)
                        rs = rs_pool.tile([P, 1], F32, name="rs")
                        nc.vector.tensor_tensor(rs, rs_h[0], rs_h[1],
                                                mybir.AluOpType.add)
                        wr = rs_pool.tile([P, P], BF16, name="wr")
                        nc.vector.reciprocal(rs, rs)
                        nc.vector.tensor_copy(
                            out=wr, in_=rs.to_broadcast([P, P]))
                        pending = (wr, e_sb, qc)
                    emit_matvec(pending)
                    # v^T *= colsum for this head's 64 partitions
                    nc.vector.tensor_tensor(
                        vt[bp:bp + 64, :], vt[bp:bp + 64, :],
                        ps_c[bp:bp + 64, :], mybir.AluOpType.mult)

            # ---- phase 4: out_partial = values @ w_o_slice  [s, hidden] ----
            with tc.tile_pool(name="osb", bufs=3) as o_pool, \
                 tc.tile_pool(name="ps_o", bufs=2, space="PSUM") as ps_o_pool:
                for sc in range(QC):
                    ps_o = ps_o_pool.tile([P, HIDDEN], F32, name="pso")
                    for nh in range(2):
                        for kc in range(2):
                            nc.tensor.matmul(
                                ps_o[:, nh * 512:(nh + 1) * 512],
                                qkvt[4 + kc][:, sc * P:(sc + 1) * P],
                                wo_t[kc][:, nh * 512:(nh + 1) * 512],
                                start=(kc == 0), stop=(kc == 1))
                    o_sb = o_pool.tile([P, HIDDEN], F32, name="osb")
                    nc.vector.tensor_copy(out=o_sb, in_=ps_o)
                    nc.sync.dma_start(out=out_d[sc * P:(sc + 1) * P, :], in_=o_sb)

    nc.compile()
    _CACHE["nc"] = nc
    return nc


def kernel(x: np.ndarray, w_qkv: np.ndarray, w_o: np.ndarray) -> np.ndarray:
    import ml_dtypes
    from concourse.bass_utils import run_bass_kernel_spmd

    nc = _build()

    xT = [round_f32r(np.ascontiguousarray(x[b].T)) for b in range(B)]
    xTb = [np.ascontiguousarray(x[b].T).astype(ml_dtypes.bfloat16) for b in range(B)]
    in_maps = []
    for c in range(N_CORES):
        b, g = divmod(c, HPC)
        wqk_slice = np.concatenate(
            [w_qkv[:, t * NH * HD + 256 * g: t * NH * HD + 256 * g + 256]
             for t in range(2)], axis=1)
        wv_slice = w_qkv[:, 2 * NH * HD + 256 * g: 2 * NH * HD + 256 * g + 256]
        wo_slice = w_o[256 * g:256 * g + 256, :]
        in_maps.append({
            "xT": xT[b],
            "xTb": xTb[b],
            "wq": round_f32r(wv_slice),
            "wqb": wqk_slice.astype(ml_dtypes.bfloat16),
            "wo": round_f32r(wo_slice),
        })

    res = run_bass_kernel_spmd(nc, in_maps, list(range(N_CORES)),
                               **_CACHE.get("run_kwargs", {}))
    _CACHE["last_result"] = res

    out = np.zeros((B, S, HIDDEN), np.float32)
    for c in range(N_CORES):
        out[c // HPC] += res.results[c]["out"]
    return out
